# revision 47
# baseline (speedup 1.0000x reference)
"""CGNN graph-diffusion kernel for Trainium2 (8 NeuronCores, SPMD data-parallel).

Math (from the reference):
    h0 = x @ fc_in_w.T + fc_in_b
    alph = sigmoid(alpha_train); dc = clip(d, 0, 1); w_eff = (w * dc) @ w.T
    repeat 8x:  h <- h + dt*(alph*0.5*(adj@h - h) + h@w_eff - h + x0),  dt = 1/8

Rewritten per step as
    h <- c1(n) * h + adj2 @ h + h @ w2 + dt * x0
with  c1(n) = 1 - dt - 0.5*dt*alph(n),  adj2 = (0.5*dt*alph(n)) * adj,  w2 = dt*w_eff.

Sharding: batch dim (32) split 4-per-core across 8 cores; adj/params replicated.
Everything lives in SBUF for all 8 steps.

Precision plan (HW-measured constraints):
  - adj matmul: fp8 e4m3 DoubleRow (2 node-tiles per pass, 1.96x over bf16).
    Quantization noise averages over the 1024-wide contraction -> harmless.
  - w2 matmul: bf16 h (stationary) x bf16 w2 (moving). h's fp8 rounding is
    correlated across steps (h changes ~15%/step) and w2 is diag-dominated,
    so fp8 here accumulates coherent error and fails the 2e-2 gate.
  - identity path c1*h + dt*x: exact fp32 h state; dt*x held in bf16.
  - PSUM carries a 2^12 product scale (adj: 2^16 on adj2 x h/16; w2: 4096 on
    w2 x h), undone in the final combine.

Per step: PRE = c1*h + dt*x on DVE during the matmul phase; one fused
combine per node-tile on the tail; shadow refreshes (fp8 node-major on ACT,
bf16 node-major on ACT) ping-pong/single-buffer per dependency needs;
feature-major bf16 shadow via PE transposes + DVE copies.
"""

import os
import sys
from contextlib import ExitStack

import numpy as np

for _p in ("/opt/trn_rl_repo", "/root/.axon_site/_ro/trn_rl_repo"):
    if os.path.isdir(_p) and _p not in sys.path:
        sys.path.insert(0, _p)

import concourse.bass as bass  # noqa: E402
import concourse.mybir as mybir  # noqa: E402
import concourse.tile as tile  # noqa: E402
from concourse import bacc  # noqa: E402
from concourse.bass_utils import run_bass_kernel_spmd  # noqa: E402
from concourse.masks import make_identity  # noqa: E402

B, N, D = 32, 1024, 256
NCORES = 8
BL = B // NCORES  # 4 batches per core
P = 128
NT = N // P  # 8 node tiles
DTl = D // P  # 2 feature tiles
NSTEP = 8
DT_C = 1.0 / NSTEP  # dt = spatial_scale / n_steps

F32 = mybir.dt.float32
BF16 = mybir.dt.bfloat16
FP8 = mybir.dt.float8e4
SAJ = 2.0**16  # scale on adj2 in fp8 (pairs with h/16 -> psum 2^12)
PSI = 1.0 / 4096.0  # psum descale
MUL = mybir.AluOpType.mult
ADD = mybir.AluOpType.add
DR = mybir.MatmulPerfMode.DoubleRow


def _body(ctx, tc, xap, adjap, alphaap, wap, dap, fcwap, fcbap, outap, nstep=NSTEP):
    nc = tc.nc

    state = ctx.enter_context(tc.tile_pool(name="state", bufs=1))
    trans = ctx.enter_context(tc.tile_pool(name="trans", bufs=1))
    const = ctx.enter_context(tc.tile_pool(name="const", bufs=1))
    # PSUM: step groups are [128,1024] f32 (2 banks) x3 bufs; transpose/init
    # tiles are one bank x2 bufs -> 8 banks total.
    pg = ctx.enter_context(tc.tile_pool(name="pg", bufs=3, space="PSUM"))
    pt = ctx.enter_context(tc.tile_pool(name="pt", bufs=2, space="PSUM"))

    # ---- persistent SBUF state ----
    HA = state.tile([P, NT, BL * D], F32, tag="HA")  # x landing, then h ping
    HB = state.tile([P, NT, BL * D], F32, tag="HB")  # h pong
    XR16 = state.tile([P, NT, BL * D], BF16, tag="XR16")  # dt*x (bf16)
    # fp8 node-major shadow (adj matmul moving operand): double-buffered so
    # step k's refresh writes B while step k's matmuls read A.
    HN8A = state.tile([P, NT, BL * D], FP8, tag="HN8A")  # h/16
    HN8B = state.tile([P, NT, BL * D], FP8, tag="HN8B")
    # bf16 feature-major shadow (w2 stationary): double-buffered likewise
    HT16A = state.tile([P, DTl, BL, N], BF16, tag="HT16A")  # h
    HT16B = state.tile([P, DTl, BL, N], BF16, tag="HT16B")
    AJT8 = state.tile([P, NT, N], FP8, tag="AJT8")  # adj2^T*2^16: [m_part, mt, n]

    # ---- transient buffers (lifetimes chained by tag reuse) ----
    ADJN = trans.tile([P, NT, N], F32, tag="bigf")  # adj rows, node-major
    ADJ8 = trans.tile([P, NT, N], FP8, tag="adj8")  # scaled adj2*2^16 fp8

    # ---- constants ----
    W28 = const.tile([P, DTl, D], BF16, tag="W28")  # dt*w_eff*4096
    WFC = const.tile([P, DTl, D], BF16, tag="WFC")  # fc_in_w.T [d_part, dt, e]
    FCWS = const.tile([P, DTl, D], F32, tag="FCWS")  # fc_in_w rows [e_part, et, d]
    FCB = const.tile([1, D], F32, tag="FCB")
    FCB16 = const.tile([1, D], BF16, tag="FCB16")
    ONES16 = const.tile([1, 512], BF16, tag="ONES16")
    C1 = const.tile([P, NT], F32, tag="C1")
    C2 = const.tile([P, NT], F32, tag="C2")
    C2S = const.tile([P, NT], F32, tag="C2S")  # c2 * 2^16
    IDT = const.tile([P, P], F32, tag="IDT")
    IDT16 = const.tile([P, P], BF16, tag="IDT16")
    IDT8 = const.tile([P, P], FP8, tag="IDT8")
    WS = const.tile([P, DTl, D], F32, tag="WS")  # w rows [i_part, it, j]
    WT = const.tile([P, DTl, D], F32, tag="WT")  # w^T [j_part, jt, i]
    WTD = const.tile([P, DTl, D], F32, tag="WTD")  # (w^T * d) [j_part, jt, i]
    DPART = const.tile([P, DTl], F32, tag="DPART")

    # ---- identity + constants ----
    make_identity(nc, IDT[:, :])
    make_identity(nc, IDT16[:, :])
    nc.vector.tensor_copy(IDT8[:, :], IDT16[:, :])
    nc.gpsimd.memset(ONES16[:, :], 1.0)

    # ---- input DMAs (small params first so PE/DVE warm up immediately) ----
    nc.sync.dma_start(out=C2[:, :], in_=alphaap.rearrange("(t p) -> p t", p=P))
    nc.sync.dma_start(out=DPART[:, :], in_=dap.rearrange("(t p) -> p t", p=P))
    for it in range(DTl):
        nc.sync.dma_start(out=WS[:, it, :], in_=wap[it * P : (it + 1) * P, :])
        nc.sync.dma_start(out=FCWS[:, it, :], in_=fcwap[it * P : (it + 1) * P, :])
    nc.sync.dma_start(out=FCB[:, :], in_=fcbap.rearrange("(o d) -> o d", o=1))
    # x rows (landing in HA) on the vector queue, adj rows on the sync
    # queue: the two setup chains (x -> XT -> fc vs adj -> scale ->
    # transpose) stream their inputs concurrently.
    for nth in range(2):
        for b in range(BL):
            nc.scalar.dma_start(
                out=HA[:, nth * 4 : (nth + 1) * 4, b * D : (b + 1) * D],
                in_=xap[b, nth * 512 : (nth + 1) * 512, :].rearrange(
                    "(t p) d -> p t d", p=P
                ),
            )
        nc.sync.dma_start(
            out=ADJN[:, nth * 4 : (nth + 1) * 4, :],
            in_=adjap[nth * 512 : (nth + 1) * 512, :].rearrange(
                "(t p) m -> p t m", p=P
            ),
        )

    # ---- scalar constants: c2 = 0.5*dt*sigmoid(alpha), c1 = (1-dt) - c2 ----
    nc.scalar.activation(C2[:, :], C2[:, :], mybir.ActivationFunctionType.Sigmoid)
    nc.vector.tensor_scalar_mul(C2[:, :], C2[:, :], 0.5 * DT_C)
    nc.vector.tensor_scalar(C1[:, :], C2[:, :], -1.0, 1.0 - DT_C, MUL, ADD)
    nc.vector.tensor_scalar_mul(C2S[:, :], C2[:, :], SAJ)

    # ---- d clamp to [0,1] ----
    nc.vector.tensor_scalar_min(DPART[:, :], DPART[:, :], 1.0)
    nc.vector.tensor_scalar_max(DPART[:, :], DPART[:, :], 0.0)

    # ---- fc bias to bf16, pre-scaled by dt (the fc matmuls run on dt*x;
    # the psum carries dt*(x@W^T + b), rescaled x8 exactly in the copies) ----
    nc.vector.tensor_scalar_mul(FCB16[:, :], FCB[:, :], DT_C)

    # ---- w^T via PE transposes; w_eff = (w*dc) @ w.T ; W28 = dt*w_eff*4096 ----
    for jt in range(DTl):
        ps = pt.tile([P, 512], F32, tag="ptr")
        for it in range(DTl):
            nc.tensor.transpose(
                ps[:, it * P : (it + 1) * P],
                WS[:, it, jt * P : (jt + 1) * P],
                IDT[:, :],
            )
        nc.vector.tensor_copy(WT[:, jt, 0 : 2 * P], ps[:, 0 : 2 * P])
        nc.vector.tensor_scalar_mul(WTD[:, jt, :], WT[:, jt, :], DPART[:, jt : jt + 1])
    for it in range(DTl):
        ps = pt.tile([P, 512], F32, tag="ptr")
        for jt in range(DTl):
            nc.tensor.matmul(
                ps[:, 0:D],
                WTD[:, jt, it * P : (it + 1) * P],
                WT[:, jt, :],
                start=(jt == 0),
                stop=(jt == DTl - 1),
            )
        nc.scalar.mul(W28[:, it, :], ps[:, 0:D], DT_C * 4096.0)

    # ---- WFC = fc_in_w.T (bf16) via PE transposes ----
    for dt_ in range(DTl):
        ps = pt.tile([P, 512], F32, tag="ptr")
        for et in range(DTl):
            nc.tensor.transpose(
                ps[:, et * P : (et + 1) * P],
                FCWS[:, et, dt_ * P : (dt_ + 1) * P],
                IDT[:, :],
            )
        nc.vector.tensor_copy(WFC[:, dt_, 0 : 2 * P], ps[:, 0 : 2 * P])

    # ---- adj2 = c2(n)*adj*2^16 in fp8, then transpose into AJT8.
    # Packed by nt so each row-block is processed as soon as its DMA lands. ----
    for nt in range(NT):
        nc.vector.tensor_scalar_mul(
            ADJ8[:, nt, :], ADJN[:, nt, :], C2S[:, nt : nt + 1]
        )
        for mtb in range(2):
            ps = pt.tile([P, 512, 2], FP8, tag="ptr")
            for j in range(4):
                mt = mtb * 4 + j
                nc.tensor.transpose(
                    ps[:, j * P : (j + 1) * P, 0],
                    ADJ8[:, nt, mt * P : (mt + 1) * P],
                    IDT8[:, :],
                )
            nc.scalar.mul(
                AJT8[:, mtb * 4 : (mtb + 1) * 4, nt * P : (nt + 1) * P],
                ps[:, :, 0].rearrange("p (m f) -> p m f", m=4),
                1.0,
            )

    # ---- XR16 <- dt*x (bf16), from raw x sitting in HA ----
    for nt in range(NT):
        nc.scalar.mul(XR16[:, nt, :], HA[:, nt, :], DT_C)

    # ---- XT (feature-major bf16 dt*x) via bf16 PE transposes of XR16, per
    # nt. Own buffer so x processing overlaps adj processing. ----
    XT = trans.tile([P, DTl, BL, N], BF16, tag="xt")
    for nt in range(NT):
        for dt_ in range(DTl):
            ps = pt.tile([P, 512], BF16, tag="ptr")
            for b in range(BL):
                nc.tensor.transpose(
                    ps[:, b * P : (b + 1) * P],
                    XR16[:, nt, b * D + dt_ * P : b * D + (dt_ + 1) * P],
                    IDT16[:, :],
                )
            nc.vector.tensor_copy(
                XT[:, dt_, :, nt * P : (nt + 1) * P],
                ps[:, :].rearrange("p (b f) -> p b f", b=BL),
            )

    # ---- fc_in: h0 node-major into HA (overwrites x) + HN8A (fp8, h/16) ----
    for nt in range(NT):
        for bp in range(2):  # batch pairs (2 per 512-wide psum bank)
            b0 = 2 * bp
            ps = pt.tile([P, 512], F32, tag="ptr")
            for b2 in range(2):
                b = b0 + b2
                sl = slice(b2 * D, (b2 + 1) * D)
                for dt_ in range(DTl):
                    nc.tensor.matmul(
                        ps[:, sl],
                        XT[:, dt_, b, nt * P : (nt + 1) * P],
                        WFC[:, dt_, :],
                        start=(b2 == 0 and dt_ == 0),
                        stop=False,
                    )
                nc.tensor.matmul(
                    ps[:, sl],
                    ONES16[0:1, 0:P],
                    FCB16[0:1, :],
                    start=False,
                    stop=(b2 == 1),
                )
            nc.vector.tensor_scalar_mul(
                HA[:, nt, b0 * D : (b0 + 2) * D], ps[:, :], 8.0
            )
            nc.scalar.mul(HN8A[:, nt, b0 * D : (b0 + 2) * D], ps[:, :], 0.5)

    # ---- fc_in: h0 feature-major (bf16) into HT16A ----
    for b in range(BL):
        for et in range(DTl):
            for nh in range(2):
                ps = pt.tile([P, 512], F32, tag="ptr")
                nsl = slice(nh * 512, (nh + 1) * 512)
                for dt_ in range(DTl):
                    nc.tensor.matmul(
                        ps[:, :],
                        WFC[:, dt_, et * P : (et + 1) * P],
                        XT[:, dt_, b, nsl],
                        start=(dt_ == 0),
                        stop=False,
                    )
                nc.tensor.matmul(
                    ps[:, :],
                    FCB16[0:1, et * P : (et + 1) * P],
                    ONES16[0:1, 0:512],
                    start=False,
                    stop=True,
                )
                nc.scalar.mul(HT16A[:, et, b, nsl], ps[:, :], 8.0)

    # ---- bf16 node-major shadow for the step transposes (single buffer:
    # its only readers are same-step transposes). Reuses ADJN's space
    # (ADJN is dead once the adj scaling into ADJ8 finishes). ----
    HN16 = trans.tile([P, NT, BL * D], BF16, tag="bigf")

    # ---- Euler steps ----
    # The feature-major shadow refresh is split into two transpose batches
    # interleaved with the NEXT step's matmul groups: batch ntb=0 (HN16
    # tiles 0-3) is emitted right after this step's first-half combines,
    # and batch ntb=1 is deferred into the next step, between its MM groups
    # 0-3 and 4-7 (just before its output range is read). PE transposes
    # don't count as busy for the HAM clock gate, so a contiguous 7us
    # transpose phase would re-throttle the PE every step; sandwiching the
    # batches between matmul bursts keeps it at full clock.
    def ht_transpose_batch(ntb, dest):
        for b in range(BL):
            ps = pt.tile([P, 1024], BF16, tag="ptr")
            for dt_ in range(DTl):
                for j in range(4):
                    nt = ntb * 4 + j
                    nc.tensor.transpose(
                        ps[:, dt_ * 512 + j * P : dt_ * 512 + (j + 1) * P],
                        HN16[:, nt, b * D + dt_ * P : b * D + (dt_ + 1) * P],
                        IDT16[:, :],
                    )
            nc.vector.tensor_copy(
                dest[:, :, b, ntb * 512 : (ntb + 1) * 512],
                ps[:, :].rearrange("p (t f) -> p t f", t=DTl),
            )

    def mm_group(nt, hs_c, ht_c, hn):
        ps = pg.tile([P, 1024], F32, tag="pgrp")
        # adj2 @ h : contract over nodes, fp8 DoubleRow (2 node-tiles
        # per pass); both 512-halves share each stationary load
        for q in range(NT // 2):
            for half in range(2):
                nc.tensor.matmul(
                    ps[:, half * 512 : (half + 1) * 512],
                    AJT8[:, 2 * q : 2 * q + 2, nt * P : (nt + 1) * P],
                    hs_c[:, 2 * q : 2 * q + 2, half * 512 : (half + 1) * 512],
                    start=(q == 0),
                    stop=False,
                    perf_mode=DR,
                )
        # h @ w2 : contract over features (bf16 x bf16)
        for b in range(BL):
            for dt_ in range(DTl):
                nc.tensor.matmul(
                    ps[:, b * D : (b + 1) * D],
                    ht_c[:, dt_, b, nt * P : (nt + 1) * P],
                    W28[:, dt_, :],
                    start=False,
                    stop=(dt_ == DTl - 1 and b % 2 == 1),
                )
        # h_new = psum/2^12 + PRE
        nc.vector.scalar_tensor_tensor(
            hn[:, nt, :], ps[:, :], PSI, hn[:, nt, :], MUL, ADD
        )

    hc, hn = HA, HB
    hs_c, hs_n = HN8A, HN8B
    ht_c, ht_n = HT16A, HT16B
    pending_ntb1 = False  # prev step's ntb=1 batch still to emit (-> ht_c)
    for step in range(nstep):
        last = step == nstep - 1
        # PRE: hn <- c1*h + dt*x on DVE, during the matmul phase.
        for nt in range(NT):
            nc.vector.scalar_tensor_tensor(
                hn[:, nt, :], hc[:, nt, :], C1[:, nt : nt + 1], XR16[:, nt, :],
                MUL, ADD,
            )
        for nt in range(NT // 2):
            mm_group(nt, hs_c, ht_c, hn)
            if not last:
                nc.scalar.mul(hs_n[:, nt, :], hn[:, nt, :], 1.0 / 16.0)
                nc.scalar.mul(HN16[:, nt, :], hn[:, nt, :], 1.0)
        if pending_ntb1:
            # completes ht_c's second half (reads prev-step HN16 tiles 4-7;
            # this step's tile 4-7 refreshes below are ordered after it)
            ht_transpose_batch(1, ht_c)
        for nt in range(NT // 2, NT):
            mm_group(nt, hs_c, ht_c, hn)
            if not last:
                nc.scalar.mul(hs_n[:, nt, :], hn[:, nt, :], 1.0 / 16.0)
                nc.scalar.mul(HN16[:, nt, :], hn[:, nt, :], 1.0)
        if not last:
            ht_transpose_batch(0, ht_n)
        pending_ntb1 = not last
        hc, hn = hn, hc
        hs_c, hs_n = hs_n, hs_c
        ht_c, ht_n = ht_n, ht_c

    # ---- store result ----
    for b in range(BL):
        for nt in range(NT):
            nc.sync.dma_start(
                out=outap[b, nt * P : (nt + 1) * P, :],
                in_=hc[:, nt, b * D : (b + 1) * D],
            )


def build(reps=1, nstep=NSTEP):
    nc = bacc.Bacc("TRN2", target_bir_lowering=False, debug=False)
    x_t = nc.dram_tensor("x", [BL, N, D], F32, kind="ExternalInput")
    adj_t = nc.dram_tensor("adj_mx", [N, N], F32, kind="ExternalInput")
    alpha_t = nc.dram_tensor("alpha_train", [N], F32, kind="ExternalInput")
    w_t = nc.dram_tensor("w", [D, D], F32, kind="ExternalInput")
    d_t = nc.dram_tensor("d", [D], F32, kind="ExternalInput")
    fcw_t = nc.dram_tensor("fc_in_w", [D, D], F32, kind="ExternalInput")
    fcb_t = nc.dram_tensor("fc_in_b", [D], F32, kind="ExternalInput")
    out_t = nc.dram_tensor("out", [BL, N, D], F32, kind="ExternalOutput")

    with tile.TileContext(nc) as tc:
        with ExitStack() as ctx:
            args = (
                ctx,
                tc,
                x_t.ap(),
                adj_t.ap(),
                alpha_t.ap(),
                w_t.ap(),
                d_t.ap(),
                fcw_t.ap(),
                fcb_t.ap(),
                out_t.ap(),
            )
            if reps == 1:
                _body(*args, nstep=nstep)
            else:
                with tc.For_i(0, reps, 1):
                    _body(*args, nstep=nstep)
    nc.compile()
    return nc


_NC = None


def _get_nc():
    global _NC
    if _NC is None:
        _NC = build()
    return _NC


def _in_maps(x, adj_mx, alpha_train, w, d, fc_in_w, fc_in_b):
    def f(a):
        return np.ascontiguousarray(np.asarray(a), dtype=np.float32)

    x = f(x)
    shared = {
        "adj_mx": f(adj_mx),
        "alpha_train": f(alpha_train),
        "w": f(w),
        "d": f(d),
        "fc_in_w": f(fc_in_w),
        "fc_in_b": f(fc_in_b),
    }
    return [
        {"x": np.ascontiguousarray(x[c * BL : (c + 1) * BL]), **shared}
        for c in range(NCORES)
    ]


def run(x, adj_mx, alpha_train, w, d, fc_in_w, fc_in_b, vt=0, **spmd_kwargs):
    nc = _get_nc()
    res = run_bass_kernel_spmd(
        nc,
        _in_maps(x, adj_mx, alpha_train, w, d, fc_in_w, fc_in_b),
        core_ids=list(range(NCORES)),
        **spmd_kwargs,
    )
    out = np.concatenate([res.results[c]["out"] for c in range(NCORES)], axis=0)
    return out, res


def kernel(x, adj_mx, alpha_train, w, d, fc_in_w, fc_in_b, vt=0):
    out, _ = run(x, adj_mx, alpha_train, w, d, fc_in_w, fc_in_b, vt)
    return out


# revision 52
# speedup vs baseline: 1.0352x; 1.0352x over previous
"""CGNN graph-diffusion kernel for Trainium2 (8 NeuronCores, SPMD data-parallel).

Math (from the reference):
    h0 = x @ fc_in_w.T + fc_in_b
    alph = sigmoid(alpha_train); dc = clip(d, 0, 1); w_eff = (w * dc) @ w.T
    repeat 8x:  h <- h + dt*(alph*0.5*(adj@h - h) + h@w_eff - h + x0),  dt = 1/8

Rewritten per step as
    h <- c1(n) * h + adj2 @ h + h @ w2 + dt * x0
with  c1(n) = 1 - dt - 0.5*dt*alph(n),  adj2 = (0.5*dt*alph(n)) * adj,  w2 = dt*w_eff.

Sharding: batch dim (32) split 4-per-core across 8 cores; adj/params replicated.
Everything lives in SBUF for all 8 steps.

Precision plan (HW-measured constraints):
  - adj matmul: fp8 e4m3 DoubleRow (2 node-tiles per pass, 1.96x over bf16).
    Quantization noise averages over the 1024-wide contraction -> harmless.
  - w2 matmul: bf16 h (stationary) x bf16 w2 (moving). h's fp8 rounding is
    correlated across steps (h changes ~15%/step) and w2 is diag-dominated,
    so fp8 here accumulates coherent error and fails the 2e-2 gate.
  - identity path c1*h + dt*x: exact fp32 h state; dt*x held in bf16.
  - PSUM carries a 2^12 product scale (adj: 2^16 on adj2 x h/16; w2: 4096 on
    w2 x h), undone in the final combine.

Per step: PRE = c1*h + dt*x on DVE during the matmul phase; one fused
combine per node-tile on the tail; shadow refreshes (fp8 node-major on ACT,
bf16 node-major on ACT) ping-pong/single-buffer per dependency needs;
feature-major bf16 shadow via PE transposes + DVE copies.
"""

import os
import sys
from contextlib import ExitStack

import numpy as np

for _p in ("/opt/trn_rl_repo", "/root/.axon_site/_ro/trn_rl_repo"):
    if os.path.isdir(_p) and _p not in sys.path:
        sys.path.insert(0, _p)

import concourse.bass as bass  # noqa: E402
import concourse.mybir as mybir  # noqa: E402
import concourse.tile as tile  # noqa: E402
from concourse import bacc  # noqa: E402
from concourse.bass_utils import run_bass_kernel_spmd  # noqa: E402
from concourse.masks import make_identity  # noqa: E402

B, N, D = 32, 1024, 256
NCORES = 8
BL = B // NCORES  # 4 batches per core
P = 128
NT = N // P  # 8 node tiles
DTl = D // P  # 2 feature tiles
NSTEP = 8
DT_C = 1.0 / NSTEP  # dt = spatial_scale / n_steps

F32 = mybir.dt.float32
BF16 = mybir.dt.bfloat16
FP8 = mybir.dt.float8e4
SAJ = 2.0**16  # scale on adj2 in fp8 (pairs with h/16 -> psum 2^12)
PSI = 1.0 / 4096.0  # psum descale
MUL = mybir.AluOpType.mult
ADD = mybir.AluOpType.add
DR = mybir.MatmulPerfMode.DoubleRow


def _body(ctx, tc, xap, adjap, alphaap, wap, dap, fcwap, fcbap, outap, nstep=NSTEP):
    nc = tc.nc

    state = ctx.enter_context(tc.tile_pool(name="state", bufs=1))
    trans = ctx.enter_context(tc.tile_pool(name="trans", bufs=1))
    const = ctx.enter_context(tc.tile_pool(name="const", bufs=1))
    # PSUM: step groups are [128,1024] f32 (2 banks) x3 bufs; transpose/init
    # tiles are one bank x2 bufs -> 8 banks total.
    pg = ctx.enter_context(tc.tile_pool(name="pg", bufs=3, space="PSUM"))
    pt = ctx.enter_context(tc.tile_pool(name="pt", bufs=2, space="PSUM"))

    # ---- persistent SBUF state ----
    HA = state.tile([P, NT, BL * D], F32, tag="HA")  # x landing, then h ping
    HB = state.tile([P, NT, BL * D], F32, tag="HB")  # h pong
    XR16 = state.tile([P, NT, BL * D], BF16, tag="XR16")  # dt*x (bf16)
    # fp8 node-major shadow (adj matmul moving operand): double-buffered so
    # step k's refresh writes B while step k's matmuls read A.
    HN8A = state.tile([P, NT, BL * D], FP8, tag="HN8A")  # h/16
    HN8B = state.tile([P, NT, BL * D], FP8, tag="HN8B")
    # bf16 feature-major shadow (w2 stationary): double-buffered likewise
    HT16A = state.tile([P, DTl, BL, N], BF16, tag="HT16A")  # h
    HT16B = state.tile([P, DTl, BL, N], BF16, tag="HT16B")
    AJT8 = state.tile([P, NT, N], FP8, tag="AJT8")  # adj2^T*2^16: [m_part, mt, n]

    # ---- transient buffers (lifetimes chained by tag reuse) ----
    ADJN = trans.tile([P, NT, N], F32, tag="bigf")  # adj rows, node-major
    ADJ8 = trans.tile([P, NT, N], FP8, tag="adj8")  # scaled adj2*2^16 fp8

    # ---- constants ----
    W28 = const.tile([P, DTl, D], BF16, tag="W28")  # dt*w_eff*4096
    WFC = const.tile([P, DTl, D], BF16, tag="WFC")  # fc_in_w.T [d_part, dt, e]
    FCWS = const.tile([P, DTl, D], F32, tag="FCWS")  # fc_in_w rows [e_part, et, d]
    FCB = const.tile([1, D], F32, tag="FCB")
    FCB16 = const.tile([1, D], BF16, tag="FCB16")
    ONES16 = const.tile([1, 512], BF16, tag="ONES16")
    C1 = const.tile([P, NT], F32, tag="C1")
    C2 = const.tile([P, NT], F32, tag="C2")
    C2S = const.tile([P, NT], F32, tag="C2S")  # c2 * 2^16
    IDT = const.tile([P, P], F32, tag="IDT")
    IDT16 = const.tile([P, P], BF16, tag="IDT16")
    IDT8 = const.tile([P, P], FP8, tag="IDT8")
    WS = const.tile([P, DTl, D], F32, tag="WS")  # w rows [i_part, it, j]
    WT = const.tile([P, DTl, D], F32, tag="WT")  # w^T [j_part, jt, i]
    WTD = const.tile([P, DTl, D], F32, tag="WTD")  # (w^T * d) [j_part, jt, i]
    DPART = const.tile([P, DTl], F32, tag="DPART")

    # ---- identity + constants ----
    make_identity(nc, IDT[:, :])
    make_identity(nc, IDT16[:, :])
    nc.vector.tensor_copy(IDT8[:, :], IDT16[:, :])
    nc.gpsimd.memset(ONES16[:, :], 1.0)

    # ---- input DMAs (small params first so PE/DVE warm up immediately) ----
    nc.sync.dma_start(out=C2[:, :], in_=alphaap.rearrange("(t p) -> p t", p=P))
    nc.sync.dma_start(out=DPART[:, :], in_=dap.rearrange("(t p) -> p t", p=P))
    for it in range(DTl):
        nc.sync.dma_start(out=WS[:, it, :], in_=wap[it * P : (it + 1) * P, :])
        nc.sync.dma_start(out=FCWS[:, it, :], in_=fcwap[it * P : (it + 1) * P, :])
    nc.sync.dma_start(out=FCB[:, :], in_=fcbap.rearrange("(o d) -> o d", o=1))
    # x rows (landing in HA) BEFORE adj rows in each half: the x chain
    # (XT transposes -> fc -> shadows) is the longest setup path, so its
    # DMA must complete first; adj processing overlaps it.
    for nth in range(2):
        for b in range(BL):
            nc.sync.dma_start(
                out=HA[:, nth * 4 : (nth + 1) * 4, b * D : (b + 1) * D],
                in_=xap[b, nth * 512 : (nth + 1) * 512, :].rearrange(
                    "(t p) d -> p t d", p=P
                ),
            )
        nc.sync.dma_start(
            out=ADJN[:, nth * 4 : (nth + 1) * 4, :],
            in_=adjap[nth * 512 : (nth + 1) * 512, :].rearrange(
                "(t p) m -> p t m", p=P
            ),
        )

    # ---- scalar constants: c2 = 0.5*dt*sigmoid(alpha), c1 = (1-dt) - c2 ----
    nc.scalar.activation(C2[:, :], C2[:, :], mybir.ActivationFunctionType.Sigmoid)
    nc.vector.tensor_scalar_mul(C2[:, :], C2[:, :], 0.5 * DT_C)
    nc.vector.tensor_scalar(C1[:, :], C2[:, :], -1.0, 1.0 - DT_C, MUL, ADD)
    nc.vector.tensor_scalar_mul(C2S[:, :], C2[:, :], SAJ)

    # ---- d clamp to [0,1] ----
    nc.vector.tensor_scalar_min(DPART[:, :], DPART[:, :], 1.0)
    nc.vector.tensor_scalar_max(DPART[:, :], DPART[:, :], 0.0)

    # ---- fc bias to bf16, pre-scaled by dt (the fc matmuls run on dt*x;
    # the psum carries dt*(x@W^T + b), rescaled x8 exactly in the copies) ----
    nc.vector.tensor_scalar_mul(FCB16[:, :], FCB[:, :], DT_C)

    # ---- w^T via PE transposes; w_eff = (w*dc) @ w.T ; W28 = dt*w_eff*4096 ----
    for jt in range(DTl):
        ps = pt.tile([P, 512], F32, tag="ptr")
        for it in range(DTl):
            nc.tensor.transpose(
                ps[:, it * P : (it + 1) * P],
                WS[:, it, jt * P : (jt + 1) * P],
                IDT[:, :],
            )
        nc.vector.tensor_copy(WT[:, jt, 0 : 2 * P], ps[:, 0 : 2 * P])
        nc.vector.tensor_scalar_mul(WTD[:, jt, :], WT[:, jt, :], DPART[:, jt : jt + 1])
    for it in range(DTl):
        ps = pt.tile([P, 512], F32, tag="ptr")
        for jt in range(DTl):
            nc.tensor.matmul(
                ps[:, 0:D],
                WTD[:, jt, it * P : (it + 1) * P],
                WT[:, jt, :],
                start=(jt == 0),
                stop=(jt == DTl - 1),
            )
        nc.scalar.mul(W28[:, it, :], ps[:, 0:D], DT_C * 4096.0)

    # ---- WFC = fc_in_w.T (bf16) via PE transposes ----
    for dt_ in range(DTl):
        ps = pt.tile([P, 512], F32, tag="ptr")
        for et in range(DTl):
            nc.tensor.transpose(
                ps[:, et * P : (et + 1) * P],
                FCWS[:, et, dt_ * P : (dt_ + 1) * P],
                IDT[:, :],
            )
        nc.vector.tensor_copy(WFC[:, dt_, 0 : 2 * P], ps[:, 0 : 2 * P])

    # ---- adj2 = c2(n)*adj*2^16 in fp8, then transpose into AJT8.
    # Packed by nt so each row-block is processed as soon as its DMA lands. ----
    for nt in range(NT):
        nc.vector.tensor_scalar_mul(
            ADJ8[:, nt, :], ADJN[:, nt, :], C2S[:, nt : nt + 1]
        )
        for mtb in range(2):
            ps = pt.tile([P, 512, 2], FP8, tag="ptr")
            for j in range(4):
                mt = mtb * 4 + j
                nc.tensor.transpose(
                    ps[:, j * P : (j + 1) * P, 0],
                    ADJ8[:, nt, mt * P : (mt + 1) * P],
                    IDT8[:, :],
                )
            nc.scalar.mul(
                AJT8[:, mtb * 4 : (mtb + 1) * 4, nt * P : (nt + 1) * P],
                ps[:, :, 0].rearrange("p (m f) -> p m f", m=4),
                1.0,
            )

    # ---- XR16 <- dt*x (bf16), from raw x sitting in HA ----
    for nt in range(NT):
        nc.scalar.mul(XR16[:, nt, :], HA[:, nt, :], DT_C)

    # ---- XT (feature-major bf16 dt*x) via bf16 PE transposes of XR16, per
    # nt. Own buffer so x processing overlaps adj processing. ----
    XT = trans.tile([P, DTl, BL, N], BF16, tag="xt")
    for nt in range(NT):
        for dt_ in range(DTl):
            ps = pt.tile([P, 512], BF16, tag="ptr")
            for b in range(BL):
                nc.tensor.transpose(
                    ps[:, b * P : (b + 1) * P],
                    XR16[:, nt, b * D + dt_ * P : b * D + (dt_ + 1) * P],
                    IDT16[:, :],
                )
            nc.vector.tensor_copy(
                XT[:, dt_, :, nt * P : (nt + 1) * P],
                ps[:, :].rearrange("p (b f) -> p b f", b=BL),
            )

    # ---- bf16 node-major shadow for the transpose batches (single buffer:
    # its only readers are same-step transposes). Reuses ADJN's space
    # (ADJN is dead once the adj scaling into ADJ8 finishes). ----
    HN16 = trans.tile([P, NT, BL * D], BF16, tag="bigf")

    # ---- fc_in: h0 node-major into HA (overwrites x) + fp8/bf16 shadows.
    # The psum carries dt*h0 (fc ran on dt*x), rescaled exactly here. ----
    for nt in range(NT):
        for bp in range(2):  # batch pairs (2 per 512-wide psum bank)
            b0 = 2 * bp
            ps = pt.tile([P, 512], F32, tag="ptr")
            for b2 in range(2):
                b = b0 + b2
                sl = slice(b2 * D, (b2 + 1) * D)
                for dt_ in range(DTl):
                    nc.tensor.matmul(
                        ps[:, sl],
                        XT[:, dt_, b, nt * P : (nt + 1) * P],
                        WFC[:, dt_, :],
                        start=(b2 == 0 and dt_ == 0),
                        stop=False,
                    )
                nc.tensor.matmul(
                    ps[:, sl],
                    ONES16[0:1, 0:P],
                    FCB16[0:1, :],
                    start=False,
                    stop=(b2 == 1),
                )
            nc.vector.tensor_scalar_mul(
                HA[:, nt, b0 * D : (b0 + 2) * D], ps[:, :], 8.0
            )
            nc.scalar.mul(HN8A[:, nt, b0 * D : (b0 + 2) * D], ps[:, :], 0.5)
            nc.scalar.mul(HN16[:, nt, b0 * D : (b0 + 2) * D], ps[:, :], 8.0)

    # ---- Euler steps ----
    # The feature-major shadow refresh is split into two transpose batches
    # interleaved with the NEXT step's matmul groups: batch ntb=0 (HN16
    # tiles 0-3) is emitted right after this step's first-half combines,
    # and batch ntb=1 is deferred into the next step, between its MM groups
    # 0-3 and 4-7 (just before its output range is read). PE transposes
    # don't count as busy for the HAM clock gate, so a contiguous 7us
    # transpose phase would re-throttle the PE every step; sandwiching the
    # batches between matmul bursts keeps it at full clock.
    def ht_transpose_batch(ntb, dest):
        for b in range(BL):
            ps = pt.tile([P, 1024], BF16, tag="ptr")
            for dt_ in range(DTl):
                for j in range(4):
                    nt = ntb * 4 + j
                    nc.tensor.transpose(
                        ps[:, dt_ * 512 + j * P : dt_ * 512 + (j + 1) * P],
                        HN16[:, nt, b * D + dt_ * P : b * D + (dt_ + 1) * P],
                        IDT16[:, :],
                    )
            nc.vector.tensor_copy(
                dest[:, :, b, ntb * 512 : (ntb + 1) * 512],
                ps[:, :].rearrange("p (t f) -> p t f", t=DTl),
            )

    def mm_group(nt, hs_c, ht_c, hn):
        ps = pg.tile([P, 1024], F32, tag="pgrp")
        # adj2 @ h : contract over nodes, fp8 DoubleRow (2 node-tiles
        # per pass); both 512-halves share each stationary load
        for q in range(NT // 2):
            for half in range(2):
                nc.tensor.matmul(
                    ps[:, half * 512 : (half + 1) * 512],
                    AJT8[:, 2 * q : 2 * q + 2, nt * P : (nt + 1) * P],
                    hs_c[:, 2 * q : 2 * q + 2, half * 512 : (half + 1) * 512],
                    start=(q == 0),
                    stop=False,
                    perf_mode=DR,
                )
        # h @ w2 : contract over features (bf16 x bf16)
        for b in range(BL):
            for dt_ in range(DTl):
                nc.tensor.matmul(
                    ps[:, b * D : (b + 1) * D],
                    ht_c[:, dt_, b, nt * P : (nt + 1) * P],
                    W28[:, dt_, :],
                    start=False,
                    stop=(dt_ == DTl - 1 and b % 2 == 1),
                )
        # h_new = psum/2^12 + PRE
        nc.vector.scalar_tensor_tensor(
            hn[:, nt, :], ps[:, :], PSI, hn[:, nt, :], MUL, ADD
        )

    hc, hn = HA, HB
    hs_c, hs_n = HN8A, HN8B
    ht_c, ht_n = HT16A, HT16B
    # bootstrap HT16A from the fc-time HN16 shadow: ntb=0 now, ntb=1
    # deferred into step 0's matmul stream (the fc feature-major matmul
    # pass this replaces was ~10us of serial setup)
    ht_transpose_batch(0, HT16A)
    pending_ntb1 = True  # ntb=1 batch still to emit (-> ht_c)
    for step in range(nstep):
        last = step == nstep - 1
        # PRE: hn <- c1*h + dt*x on DVE, during the matmul phase.
        for nt in range(NT):
            nc.vector.scalar_tensor_tensor(
                hn[:, nt, :], hc[:, nt, :], C1[:, nt : nt + 1], XR16[:, nt, :],
                MUL, ADD,
            )
        for nt in range(NT // 2):
            mm_group(nt, hs_c, ht_c, hn)
            if not last:
                nc.scalar.mul(hs_n[:, nt, :], hn[:, nt, :], 1.0 / 16.0)
                nc.scalar.mul(HN16[:, nt, :], hn[:, nt, :], 1.0)
        if pending_ntb1:
            # completes ht_c's second half (reads prev-step HN16 tiles 4-7;
            # this step's tile 4-7 refreshes below are ordered after it)
            ht_transpose_batch(1, ht_c)
        for nt in range(NT // 2, NT):
            mm_group(nt, hs_c, ht_c, hn)
            if not last:
                nc.scalar.mul(hs_n[:, nt, :], hn[:, nt, :], 1.0 / 16.0)
                nc.scalar.mul(HN16[:, nt, :], hn[:, nt, :], 1.0)
        if not last:
            ht_transpose_batch(0, ht_n)
        pending_ntb1 = not last
        hc, hn = hn, hc
        hs_c, hs_n = hs_n, hs_c
        ht_c, ht_n = ht_n, ht_c

    # ---- store result ----
    for b in range(BL):
        for nt in range(NT):
            nc.sync.dma_start(
                out=outap[b, nt * P : (nt + 1) * P, :],
                in_=hc[:, nt, b * D : (b + 1) * D],
            )


def build(reps=1, nstep=NSTEP):
    nc = bacc.Bacc("TRN2", target_bir_lowering=False, debug=False)
    x_t = nc.dram_tensor("x", [BL, N, D], F32, kind="ExternalInput")
    adj_t = nc.dram_tensor("adj_mx", [N, N], F32, kind="ExternalInput")
    alpha_t = nc.dram_tensor("alpha_train", [N], F32, kind="ExternalInput")
    w_t = nc.dram_tensor("w", [D, D], F32, kind="ExternalInput")
    d_t = nc.dram_tensor("d", [D], F32, kind="ExternalInput")
    fcw_t = nc.dram_tensor("fc_in_w", [D, D], F32, kind="ExternalInput")
    fcb_t = nc.dram_tensor("fc_in_b", [D], F32, kind="ExternalInput")
    out_t = nc.dram_tensor("out", [BL, N, D], F32, kind="ExternalOutput")

    with tile.TileContext(nc) as tc:
        with ExitStack() as ctx:
            args = (
                ctx,
                tc,
                x_t.ap(),
                adj_t.ap(),
                alpha_t.ap(),
                w_t.ap(),
                d_t.ap(),
                fcw_t.ap(),
                fcb_t.ap(),
                out_t.ap(),
            )
            if reps == 1:
                _body(*args, nstep=nstep)
            else:
                with tc.For_i(0, reps, 1):
                    _body(*args, nstep=nstep)
    nc.compile()
    return nc


_NC = None


def _get_nc():
    global _NC
    if _NC is None:
        _NC = build()
    return _NC


def _in_maps(x, adj_mx, alpha_train, w, d, fc_in_w, fc_in_b):
    def f(a):
        return np.ascontiguousarray(np.asarray(a), dtype=np.float32)

    x = f(x)
    shared = {
        "adj_mx": f(adj_mx),
        "alpha_train": f(alpha_train),
        "w": f(w),
        "d": f(d),
        "fc_in_w": f(fc_in_w),
        "fc_in_b": f(fc_in_b),
    }
    return [
        {"x": np.ascontiguousarray(x[c * BL : (c + 1) * BL]), **shared}
        for c in range(NCORES)
    ]


def run(x, adj_mx, alpha_train, w, d, fc_in_w, fc_in_b, vt=0, **spmd_kwargs):
    nc = _get_nc()
    res = run_bass_kernel_spmd(
        nc,
        _in_maps(x, adj_mx, alpha_train, w, d, fc_in_w, fc_in_b),
        core_ids=list(range(NCORES)),
        **spmd_kwargs,
    )
    out = np.concatenate([res.results[c]["out"] for c in range(NCORES)], axis=0)
    return out, res


def kernel(x, adj_mx, alpha_train, w, d, fc_in_w, fc_in_b, vt=0):
    out, _ = run(x, adj_mx, alpha_train, w, d, fc_in_w, fc_in_b, vt)
    return out
